# revision 1
# baseline (speedup 1.0000x reference)
"""Correlation cost-volume kernel for Trainium2 (8 NeuronCores).

Problem: out[b, k, i, j] = sum_c feat1[b,c,i,j] * pad(feat2)[b,c,i+dx,j+dy]
with (dx, dy) in {0,2,...,40}^2 (441 offsets), feat [4,128,96,192] fp32.

Strategy:
  - Shard (batch b, H-half) across 8 cores; feat2 slices carry the +-20 row halo.
  - Host pre-pads feat2 spatially, parity-packs the W axis (displacements are
    even, so even/odd pixel columns never mix), and casts to fp16.
  - On device, for each padded row r the PE computes band blocks:
    stationary operand = 52-col segment of the f2p row (M=52), moving operand
    = feat1 pixels of every output row i with i+dx == r (N = 32 * n_rows).
    Two parities stack in one PSUM tile at partition bases 0/64.
    psum[n, (i, p)] = corr(pixel (i, m0+p), f2p col (r, m0+n)); the useful
    entries are the diagonal band n-p in [0, 20] - extracted on the host.
  - PSUM -> SBUF evacuation casts fp32 -> fp16 (split across DVE and ACT),
    and the dense band ships to DRAM; the host shears the parallelogram out
    with numpy as_strided and assembles the [B, 441, H, W] fp32 output.
"""

import sys

if "/opt/trn_rl_repo" not in sys.path:
    sys.path.insert(0, "/opt/trn_rl_repo")

import numpy as np

B, C, H, W = 4, 128, 96, 192
D = 20           # spatial pad
ND = 21          # displacements per axis
HS = H // 2      # output rows per core
RP = HS + 2 * D  # padded rows per core (88)
PW = W // 2      # pixels per parity (96)
WPAD = (W + 2 * D) // 2  # parity-space padded width (116)
N_CORES = 8
SEG = 52         # stationary segment width (32 pixels + 20 dy)
CHUNK_ROWS = 11  # f2p rows per input DMA chunk
STAGE_COLS = 8064

_compiled = None


def gen_phases():
    """Per padded row r: the valid output rows and bank-aligned i-chunks.

    Returns (phases, TC). Each phase is (r, ipar, ih_lo, v, cum, chunks),
    chunks a list of (q, ih, vc, off) where off is the column offset inside
    the row's PSUM tile. Chunks never cross a 512-col PSUM bank boundary.
    """
    phases = []
    cum = 0
    for r in range(RP):
        ipar = r % 2
        i_min, i_max = max(0, r - 2 * D), min(HS - 1, r)
        if i_min % 2 != ipar:
            i_min += 1
        if i_max % 2 != ipar:
            i_max -= 1
        if i_min > i_max:
            continue
        ih_lo, ih_hi = i_min // 2, i_max // 2
        v = ih_hi - ih_lo + 1
        chunks = []
        off = 0
        for q in range(3):
            rem, ih = v, ih_lo
            while rem:
                space = (512 - (off % 512)) // 32
                vc = min(rem, 16, space)
                chunks.append((q, ih, vc, off))
                off += 32 * vc
                ih += vc
                rem -= vc
        phases.append((r, ipar, ih_lo, v, cum, chunks))
        cum += 96 * v
    return phases, cum


PHASES, TC = gen_phases()


def _build_module(reps=1):
    from contextlib import ExitStack, nullcontext

    import concourse.bacc as bacc
    import concourse.mybir as mybir
    import concourse.tile as tile

    fp16 = mybir.dt.float16
    fp32 = mybir.dt.float32

    nc = bacc.Bacc("TRN2", target_bir_lowering=False, debug=False,
                   enable_asserts=False, num_devices=N_CORES)
    w_ap = nc.dram_tensor("w", [C, 2, HS // 2, 2, PW], fp16,
                          kind="ExternalInput").ap()
    x_ap = nc.dram_tensor("x", [C, RP, 2, WPAD], fp16,
                          kind="ExternalInput").ap()
    d_ap = nc.dram_tensor("d", [116, TC], fp16, kind="ExternalOutput").ap()

    n_xchunks = RP // CHUNK_ROWS

    # batch rows into output-DMA groups
    batches = []
    cur, cur_cols = [], 0
    for ph in PHASES:
        cols = 96 * ph[3]
        if cur and cur_cols + cols > STAGE_COLS:
            batches.append((cur, cur_cols))
            cur, cur_cols = [], 0
        cur.append(ph)
        cur_cols += cols
    if cur:
        batches.append((cur, cur_cols))

    with tile.TileContext(nc) as tc:
        with ExitStack() as ctx:
            inp = ctx.enter_context(tc.tile_pool(name="inp", bufs=1))
            psum = ctx.enter_context(tc.tile_pool(name="psum", bufs=2, space="PSUM"))
            stg = ctx.enter_context(tc.tile_pool(name="stg", bufs=3))

            loop = (tc.For_i(0, reps, 1,
                             hint_engines=(mybir.EngineType.PE,))
                    if reps > 1 else nullcontext())
            ctx.enter_context(loop)

            w1t = inp.tile([C, 2, HS // 2, 2, PW], fp16, tag="w1")
            nc.sync.dma_start(w1t[:], w_ap[:])
            xts = []
            for xc in range(n_xchunks):
                xt = inp.tile([C, CHUNK_ROWS, 2, WPAD], fp16, tag=f"x{xc}", bufs=1)
                nc.sync.dma_start(xt[:], x_ap[:, xc * CHUNK_ROWS:(xc + 1) * CHUNK_ROWS])
                xts.append(xt)

            out_cum = 0
            for batch, bcols in batches:
                stage = stg.tile([116, STAGE_COLS], fp16, tag="stage")
                scol = 0
                for (r, ipar, ih_lo, v, cum, chunks) in batch:
                    ps = psum.tile([128, 2048], fp32, tag="ps")
                    xt = xts[r // CHUNK_ROWS]
                    lr = r % CHUNK_ROWS
                    for (q, ih, vc, off) in chunks:
                        for par in (0, 1):
                            nc.tensor.matmul(
                                ps[64 * par:64 * par + SEG, off:off + 32 * vc],
                                xt[:, lr, par, 32 * q:32 * q + SEG],
                                w1t[:, ipar, ih:ih + vc, par, 32 * q:32 * q + 32],
                                start=True, stop=True,
                                tile_position=(0, 64 * par),
                            )
                    rc = 96 * v
                    half = (rc // 2) // 32 * 32
                    nc.vector.tensor_copy(stage[:, scol:scol + half],
                                          ps[0:116, 0:half])
                    nc.scalar.copy(stage[:, scol + half:scol + rc],
                                   ps[0:116, half:rc])
                    scol += rc
                nc.sync.dma_start(d_ap[:, out_cum:out_cum + bcols],
                                  stage[:, 0:bcols])
                out_cum += bcols
    nc.compile()
    return nc


def _get_compiled():
    global _compiled
    if _compiled is None:
        _compiled = _build_module()
    return _compiled


def _prep_inputs(feat1, feat2):
    """Slice/pad/pack per-core inputs. Returns list of input dicts."""
    f2pad = np.pad(feat2, ((0, 0), (0, 0), (D, D), (D, D)))
    in_maps = []
    for core in range(N_CORES):
        b, half = divmod(core, 2)
        i0 = half * HS
        f1 = feat1[b, :, i0:i0 + HS, :]                       # [C, 48, 192]
        # (C, ih, ipar, m, par): i = 2*ih + ipar, j = 2*m + par
        w1 = (f1.reshape(C, HS // 2, 2, PW, 2)
                .transpose(0, 2, 1, 4, 3)                     # (C, ipar, ih, par, m)
                .astype(np.float16).copy())
        f2 = f2pad[b, :, i0:i0 + RP, :]                       # [C, 88, 232]
        x = (f2.reshape(C, RP, WPAD, 2)
               .transpose(0, 1, 3, 2)                         # (C, r, par, t)
               .astype(np.float16).copy())
        in_maps.append({"w": w1, "x": x})
    return in_maps


def _assemble(results):
    """Host shear: dense band blocks -> [B, 441, H, W] fp32."""
    out = np.empty((B, ND * ND, H, W), np.float32)
    # (dxi, dy, i, m, par) view of each batch image
    T = out.reshape(B, ND, ND, H, PW, 2)
    DY = np.arange(ND)
    for core in range(N_CORES):
        b, half = divmod(core, 2)
        i0 = half * HS
        Dc = np.ascontiguousarray(results[core]["d"].astype(np.float32))
        st_n, st_c = Dc.strides  # elements: (TC, 1) * 4 bytes
        for (r, ipar, ih_lo, v, cum, chunks) in PHASES:
            for (q, ih, vc, off) in chunks:
                c0 = cum + off
                I = 2 * (ih + np.arange(vc)) + ipar
                DXI = (r - I) // 2
                for par in (0, 1):
                    base = 64 * par
                    # V[dy, ci, p] = Dc[base + p + dy, c0 + ci*32 + p]
                    V = np.lib.stride_tricks.as_strided(
                        Dc[base:, c0:],
                        shape=(ND, vc, 32),
                        strides=(st_n, 32 * st_c, st_n + st_c),
                    )
                    T[b,
                      DXI[:, None, None],
                      DY[None, :, None],
                      (i0 + I)[:, None, None],
                      (32 * q + np.arange(32))[None, None, :],
                      par] = V.transpose(1, 0, 2)
    return out


def kernel(feat1, feat2):
    from concourse.bass_utils import run_bass_kernel_spmd

    feat1 = np.asarray(feat1, dtype=np.float32)
    feat2 = np.asarray(feat2, dtype=np.float32)
    nc = _get_compiled()
    in_maps = _prep_inputs(feat1, feat2)
    res = run_bass_kernel_spmd(nc, in_maps, list(range(N_CORES)))
    return _assemble(res.results)

